# revision 1
# baseline (speedup 1.0000x reference)
"""Trainium2 Bass kernel for a dense transformer block (pre-LN, causal MHA + GELU FFN).

Sharding: 8 cores = 4 batches x 2 roles. Each core handles one batch.
The two cores of a batch split the 2048 queries in a zigzag: role 0 owns
blocks [0:512) and [1536:2048), role 1 owns [512:1536). Both cores
redundantly compute LN1 + K/V for all 2048 tokens of their batch, which
avoids all cross-core communication. The causal structure is padded to a
common shape (8 k-tiles for the low query chunk, 16 for the high chunk)
and the per-role causal masks are host-provided data, so a single SPMD
program serves all cores.
"""

import time

import numpy as np
import ml_dtypes

import concourse.bass as bass
import concourse.tile as tile
from concourse import bacc
from concourse import mybir
from concourse.bass_utils import run_bass_kernel_spmd

F32 = mybir.dt.float32
BF16 = mybir.dt.bfloat16
AF = mybir.ActivationFunctionType
OP = mybir.AluOpType

B, S, E, H, DH = 4, 2048, 1024, 16, 64
MFF = 6 * E            # 6144
SO = S // 2            # own tokens per core: 1024
LN_EPS = 1e-5
NT = S // 128          # 16 token tiles (global)
NTO = SO // 128        # 8 own token tiles
NE = E // 128          # 8 feature chunks
NM = MFF // 128        # 48 ffn chunks
QC_KTILES = (8, 16)    # padded k-tile extents for the two query chunks

# debug toggles for HW bisection
import os
USE_PBCAST = True      # partition_broadcast + normalize in attention
MASK_ENGINE = "gpsimd"  # or "vector"
W2_DEPTH = int(os.environ.get("W2_DEPTH", "48"))


_prog_cache = {}


def _build_program(stage=4, reps=1):
    nc = bacc.Bacc(None)

    xg = nc.declare_dram_parameter("xg", [S, E], F32, isOutput=False)
    xo = nc.declare_dram_parameter("xo", [SO, E], F32, isOutput=False)
    wq = nc.declare_dram_parameter("wq", [E, E], BF16, isOutput=False)
    wk = nc.declare_dram_parameter("wk", [E, E], BF16, isOutput=False)
    wv = nc.declare_dram_parameter("wv", [E, E], BF16, isOutput=False)
    wo = nc.declare_dram_parameter("wo", [E, E], BF16, isOutput=False)
    w1 = nc.declare_dram_parameter("w1", [E, MFF], BF16, isOutput=False)
    w2 = nc.declare_dram_parameter("w2", [MFF, E], BF16, isOutput=False)
    bqk = nc.declare_dram_parameter("bqk", [128, 2, NE], F32, isOutput=False)
    b1d = nc.declare_dram_parameter("b1d", [128, NM], F32, isOutput=False)
    msk = nc.declare_dram_parameter("msk", [128, 8, 2048], BF16, isOutput=False)
    idn = nc.declare_dram_parameter("idn", [128, 128], BF16, isOutput=False)
    out = nc.declare_dram_parameter("out", [SO, E], F32, isOutput=True)

    with tile.TileContext(nc) as tc:
        def _body():
            # ---- kernel-wide pools ----
            gp = tc.alloc_tile_pool(name="gp", bufs=1)
            xin = tc.alloc_tile_pool(name="xin", bufs=2)
            stats = tc.alloc_tile_pool(name="stats", bufs=6)
            hrow = tc.alloc_tile_pool(name="hrow", bufs=2)

            masks = gp.tile([128, 8, 2048], BF16, tag="masks")
            ident = gp.tile([128, 128], BF16, tag="ident")
            bqk_s = gp.tile([128, 2, NE], F32, tag="bqk")
            b1_s = gp.tile([128, NM], F32, tag="b1")
            eps_t = gp.tile([128, 1], F32, tag="eps")

            nc.gpsimd.dma_start(out=ident, in_=idn[:, :])
            nc.gpsimd.dma_start(out=masks, in_=msk[:, :, :])
            nc.gpsimd.dma_start(out=bqk_s, in_=bqk[:, :, :])
            nc.gpsimd.dma_start(out=b1_s, in_=b1d[:, :])
            nc.vector.memset(eps_t, LN_EPS)

            dramp = tc.alloc_tile_pool(name="dramp", bufs=1, space="DRAM")

            def layernorm_tiles(src, ntiles, dstF, ps_tp, from_sbuf=False, dname="hd"):
                # LN per 128-token tile, spill normalized bf16 rows to DRAM,
                # then reload feature-major via DMA transpose (one per e-chunk).
                hd = dramp.tile([ntiles * 128, E], BF16, tag=dname, name=dname)
                for t in range(ntiles):
                    if from_sbuf:
                        xt = src[:, t, :]
                    else:
                        xt = xin.tile([128, E], F32, tag="xt", name=f"xt{t}")
                        nc.gpsimd.dma_start(out=xt, in_=src[t * 128:(t + 1) * 128, :])
                    st = stats.tile([128, 2, 6], F32, tag="st", name=f"st{t}")
                    nc.vector.bn_stats(out=st[:, 0, :], in_=xt[:, 0:512])
                    nc.vector.bn_stats(out=st[:, 1, :], in_=xt[:, 512:1024])
                    mv = stats.tile([128, 2], F32, tag="mv", name=f"mv{t}")
                    nc.vector.bn_aggr(out=mv, in_=st)
                    sd = stats.tile([128, 1], F32, tag="sd", name=f"sd{t}")
                    nc.scalar.activation(out=sd, in_=mv[:, 1:2], func=AF.Sqrt,
                                         bias=eps_t, scale=1.0)
                    rs = stats.tile([128, 1], F32, tag="rs", name=f"rs{t}")
                    nc.vector.reciprocal(out=rs, in_=sd)
                    ht = hrow.tile([128, E], BF16, tag="ht", name=f"ht{t}")
                    nc.vector.tensor_scalar(out=ht, in0=xt, scalar1=mv[:, 0:1],
                                            scalar2=rs, op0=OP.subtract, op1=OP.mult)
                    nc.gpsimd.dma_start(out=hd[t * 128:(t + 1) * 128, :], in_=ht)
                for e in range(NE):
                    nc.sync.dma_start(out=dstF[:, e, :],
                                      in_=hd[:, e * 128:(e + 1) * 128], transpose=True)

            # ============ phase A: LN1 + Q/K/V projections ============
            ab = tc.alloc_tile_pool(name="ab", bufs=1)
            KF = ab.tile([128, NE, S], BF16, tag="KF")
            QF = ab.tile([128, NE, SO], BF16, tag="QF")
            VT = ab.tile([128, NT, H * 65], BF16, tag="VT")

            ap = tc.alloc_tile_pool(name="ap", bufs=1)
            hF = ap.tile([128, NE, S], BF16, tag="hF")
            hFq = ap.tile([128, NE, SO], BF16, tag="hFq")
            wv_s = ap.tile([128, NE, E], BF16, tag="wv")
            wqkp = tc.alloc_tile_pool(name="wqkp", bufs=3)
            ps_tp = tc.alloc_tile_pool(name="ps_tp_a", bufs=2, space="PSUM")
            ps_mm = tc.alloc_tile_pool(name="ps_mm_a", bufs=6, space="PSUM")

            layernorm_tiles(xg, NT, hF, ps_tp, dname="hd1")
            layernorm_tiles(xo, NTO, hFq, ps_tp, dname="hdq")

            def proj_qk(w_dram, srcF, ntok, dstF, bias_col, pname):
                nch = ntok // 512
                for hp in range(NE):
                    pss = [ps_mm.tile([128, 512], F32, tag="mm", name=f"{pname}{hp}_{c}")
                           for c in range(nch)]
                    wt = wqkp.tile([128, NE, 128], BF16, tag="wqk",
                                   name=f"w{pname}{hp}")
                    nc.gpsimd.dma_start(
                        out=wt,
                        in_=w_dram[:, hp * 128:(hp + 1) * 128].rearrange(
                            "(e p) m -> p e m", p=128))
                    for e in range(NE):
                        for c in range(nch):
                            nc.tensor.matmul(
                                pss[c], wt[:, e, :], srcF[:, e, c * 512:(c + 1) * 512],
                                start=(e == 0), stop=(e == NE - 1))
                    for c in range(nch):
                        nc.any.tensor_scalar(
                            out=dstF[:, hp, c * 512:(c + 1) * 512], in0=pss[c],
                            scalar1=bqk_s[:, bias_col, hp:hp + 1], scalar2=None,
                            op0=OP.add)

            proj_qk(wq, hFq, SO, QF, 0, "q")
            proj_qk(wk, hF, S, KF, 1, "k")

            # V projection: token-major with a ones column per head
            for e in range(NE):
                nc.gpsimd.dma_start(out=wv_s[:, e, :], in_=wv[e * 128:(e + 1) * 128, :])
            VTv = VT.rearrange("p t (h c) -> p t h c", c=65)
            for t in range(NT):
                nc.vector.memset(VTv[:, t, :, 64:65], 1.0)
                for c in range(2):
                    ps = ps_mm.tile([128, 512], F32, tag="mm", name=f"v{t}_{c}")
                    for e in range(NE):
                        nc.tensor.matmul(
                            ps, hF[:, e, t * 128:(t + 1) * 128],
                            wv_s[:, e, c * 512:(c + 1) * 512],
                            start=(e == 0), stop=(e == NE - 1))
                    nc.any.tensor_copy(
                        out=VTv[:, t, 8 * c:8 * c + 8, 0:64],
                        in_=ps.rearrange("p (h c) -> p h c", c=64))

            ps_mm.release()
            ps_tp.release()
            wqkp.release()
            ap.release()

            # ============ phase B: attention ============
            skipB = stage < 2
            skipC = stage < 3
            skipD = stage < 4
            ct2p = tc.alloc_tile_pool(name="ct2p", bufs=1, side="right")
            CT2 = ct2p.tile([128, NE, SO], BF16, tag="CT2")
            if skipB:
                nc.vector.memset(CT2[:, :, :], 0.0)
            ptile = tc.alloc_tile_pool(name="ptile", bufs=3)
            small = tc.alloc_tile_pool(name="small", bufs=3)
            ps_sc = tc.alloc_tile_pool(name="ps_sc", bufs=1, space="PSUM")
            ps_ctx = tc.alloc_tile_pool(name="ps_ctx", bufs=4, space="PSUM")

            for hp in range(NE if not skipB else 0):
                for qc in range(2):
                    nkt = QC_KTILES[qc]
                    ng = nkt // 2  # groups of (2 k-tiles x 2 heads)
                    ctxs = (ps_ctx.tile([65, 512], F32, tag="ctx", name=f"cx{hp}_{qc}_0"),
                            ps_ctx.tile([65, 512], F32, tag="ctx", name=f"cx{hp}_{qc}_1"))
                    for g in range(ng):
                        sc = ps_sc.tile([128, 2048], F32, tag="sc", name=f"sc{hp}_{qc}_{g}")
                        for hh in range(2):
                            hoff = hh * 64
                            for kl in range(2):
                                kt = g * 2 + kl
                                nc.tensor.matmul(
                                    sc[:, (hh * 2 + kl) * 512:(hh * 2 + kl + 1) * 512],
                                    KF[hoff:hoff + 64, hp, kt * 128:(kt + 1) * 128],
                                    QF[hoff:hoff + 64, hp, qc * 512:(qc + 1) * 512],
                                    start=True, stop=True)
                        pt = ptile.tile([128, 2048], BF16, tag="pt", name=f"pt{hp}_{qc}_{g}")
                        nc.scalar.activation(out=pt, in_=sc, func=AF.Exp, scale=0.125)
                        # qc0: k-tiles 0..7 all need masks; qc1: only k-tiles
                        # 8..15 (groups 4..7) do.
                        if qc == 0 or g >= 4:
                            sub = g if qc == 0 else g - 4
                            slot = (0 if qc == 0 else 4) + sub
                            meng = nc.gpsimd if MASK_ENGINE == "gpsimd" else nc.vector
                            meng.tensor_tensor(out=pt, in0=pt,
                                               in1=masks[:, slot, :], op=OP.mult)
                        for hh in range(2):
                            h = hp * 2 + hh
                            for kl in range(2):
                                kt = g * 2 + kl
                                nc.tensor.matmul(
                                    ctxs[hh], VTv[:, kt, h, :],
                                    pt[:, (hh * 2 + kl) * 512:(hh * 2 + kl + 1) * 512],
                                    start=(g == 0 and kl == 0),
                                    stop=(g == ng - 1 and kl == 1))
                    for hh in range(2):
                        if USE_PBCAST:
                            rs1 = small.tile([1, 512], F32, tag="rs1", name=f"r{hp}_{qc}_{hh}")
                            nc.vector.reciprocal(out=rs1, in_=ctxs[hh][64:65, :])
                            rsb = small.tile([64, 512], F32, tag="rsb", name=f"rb{hp}_{qc}_{hh}")
                            nc.gpsimd.partition_broadcast(rsb, rs1)
                            nc.vector.tensor_tensor(
                                out=CT2[hh * 64:hh * 64 + 64, hp, qc * 512:(qc + 1) * 512],
                                in0=ctxs[hh][0:64, :], in1=rsb, op=OP.mult)
                        else:
                            nc.vector.tensor_copy(
                                out=CT2[hh * 64:hh * 64 + 64, hp, qc * 512:(qc + 1) * 512],
                                in_=ctxs[hh][0:64, :])

            ps_ctx.release()
            ps_sc.release()
            small.release()
            ptile.release()
            ab.release()

            # ============ phase C: Wo + residual, LN2, FFN up + gelu ============
            x2p = tc.alloc_tile_pool(name="x2p", bufs=1)
            X2 = x2p.tile([128, NTO, E], F32, tag="X2")
            wop = tc.alloc_tile_pool(name="wop", bufs=1)
            wo_s = wop.tile([128, NE, E], BF16, tag="wo")
            ps_tp2 = tc.alloc_tile_pool(name="ps_tp_c", bufs=2, space="PSUM")
            ps_mm2 = tc.alloc_tile_pool(name="ps_mm_c", bufs=4, space="PSUM")

            for e in range(NE):
                nc.gpsimd.dma_start(out=wo_s[:, e, :], in_=wo[e * 128:(e + 1) * 128, :])
            for qt in range(NTO):
                xot = xin.tile([128, E], F32, tag="xt", name=f"xo{qt}")
                nc.gpsimd.dma_start(out=xot, in_=xo[qt * 128:(qt + 1) * 128, :])
                for eo in range(2):
                    ps = ps_mm2.tile([128, 512], F32, tag="mm", name=f"o{qt}_{eo}")
                    for hp in range(NE):
                        nc.tensor.matmul(ps, CT2[:, hp, qt * 128:(qt + 1) * 128],
                                         wo_s[:, hp, eo * 512:(eo + 1) * 512],
                                         start=(hp == 0), stop=(hp == NE - 1))
                    nc.vector.tensor_tensor(
                        out=X2[:, qt, eo * 512:(eo + 1) * 512], in0=ps,
                        in1=xot[:, eo * 512:(eo + 1) * 512], op=OP.add)

            wop.release()
            ct2p.release()

            h2p = tc.alloc_tile_pool(name="h2p", bufs=1, side="right")
            h2F = h2p.tile([128, NE, SO], BF16, tag="h2F")
            if skipC:
                nc.vector.memset(h2F[:, :, :], 0.0)
            else:
                layernorm_tiles(X2, NTO, h2F, ps_tp2, from_sbuf=True, dname="hd2")

            hidp = tc.alloc_tile_pool(name="hidp", bufs=1)
            HID = hidp.tile([128, NM, SO], BF16, tag="HID")
            w1p = tc.alloc_tile_pool(name="w1p", bufs=3)
            for mo in range(NM if not skipC else 0):
                w1t = w1p.tile([128, NE, 128], BF16, tag="w1t", name=f"w1t{mo}")
                nc.gpsimd.dma_start(
                    out=w1t,
                    in_=w1[:, mo * 128:(mo + 1) * 128].rearrange("(e p) m -> p e m", p=128))
                for c in range(2):
                    ps = ps_mm2.tile([128, 512], F32, tag="mm", name=f"h{mo}_{c}")
                    for e in range(NE):
                        nc.tensor.matmul(
                            ps, w1t[:, e, :], h2F[:, e, c * 512:(c + 1) * 512],
                            start=(e == 0), stop=(e == NE - 1))
                    nc.scalar.activation(
                        out=HID[:, mo, c * 512:(c + 1) * 512], in_=ps, func=AF.Gelu,
                        bias=b1_s[:, mo:mo + 1], scale=1.0)

            w1p.release()
            h2p.release()
            ps_mm2.release()
            ps_tp2.release()

            # ============ phase D: FFN down + residual + store ============
            w2p = tc.alloc_tile_pool(name="w2p", bufs=3)
            outp = tc.alloc_tile_pool(name="outp", bufs=3)
            ps_f2 = tc.alloc_tile_pool(name="ps_f2", bufs=8, space="PSUM")
            if skipC or skipD:
                nc.vector.memset(HID[:, :, :], 0.0)
            for eo in range(2):
                pss = [ps_f2.tile([128, 512], F32, tag="f2", name=f"f{eo}_{j}")
                       for j in range(8)]
                for m in range((W2_DEPTH if not skipD else 1)):
                    w2t = w2p.tile([128, 512], BF16, tag="w2t", name=f"w2t{eo}_{m}")
                    nc.gpsimd.dma_start(
                        out=w2t, in_=w2[m * 128:(m + 1) * 128, eo * 512:(eo + 1) * 512])
                    for qt in range(8):
                        nc.tensor.matmul(
                            pss[qt], HID[:, m, qt * 128:(qt + 1) * 128], w2t,
                            start=(m == 0), stop=(m == NM - 1))
                for qt in range(8):
                    ot = outp.tile([128, 512], F32, tag="ot", name=f"ot{eo}_{qt}")
                    nc.vector.tensor_tensor(
                        out=ot, in0=pss[qt],
                        in1=X2[:, qt, eo * 512:(eo + 1) * 512], op=OP.add)
                    nc.gpsimd.dma_start(
                        out=out[qt * 128:(qt + 1) * 128, eo * 512:(eo + 1) * 512],
                        in_=ot)

            ps_f2.release()
            outp.release()
            w2p.release()
            hidp.release()
            x2p.release()
            hrow.release()
            stats.release()
            xin.release()
            gp.release()

        for _rep in range(reps):
            _body()

    nc.compile()
    return nc


def _own_slices(role):
    if role == 0:
        return [(0, 512), (1536, 2048)]
    return [(512, 1024), (1024, 1536)]


def _make_masks(role):
    """[128, 8, 2048] bf16; slot = qc*4 + k-tile-pair index; the pair's
    [128, 1024] mask is duplicated in both halves (one per head)."""
    qstarts = (0, 1536) if role == 0 else (512, 1024)
    m = np.zeros((128, 8, 2048), np.float32)
    ki = np.arange(128)[:, None]
    qi = np.arange(512)[None, :]
    for qc in range(2):
        qs = qstarts[qc]
        kt0 = 0 if qc == 0 else 8
        for sub in range(4):
            slot = qc * 4 + sub
            for kl in range(2):
                kt = kt0 + sub * 2 + kl
                blk = ((kt * 128 + ki) <= (qs + qi))
                m[:, slot, kl * 512:(kl + 1) * 512] = blk
                m[:, slot, 1024 + kl * 512:1024 + (kl + 1) * 512] = blk
    return m.astype(ml_dtypes.bfloat16)


def _prep_core_inputs(x, Wq, Wk, Wv, Wo, W1, W2, ln1_g, ln1_b, ln2_g, ln2_b):
    bf = ml_dtypes.bfloat16
    WqA = np.transpose(np.asarray(Wq, np.float32), (1, 0, 2)).reshape(E, E)
    WkA = np.transpose(np.asarray(Wk, np.float32), (1, 0, 2)).reshape(E, E)
    WvA = np.transpose(np.asarray(Wv, np.float32), (1, 0, 2)).reshape(E, E)
    g1 = np.asarray(ln1_g, np.float32)
    b1v = np.asarray(ln1_b, np.float32)
    g2 = np.asarray(ln2_g, np.float32)
    b2v = np.asarray(ln2_b, np.float32)
    assert np.all(b1v == 0.0), "nonzero ln1 bias unsupported (V bias path)"
    wq_d = (g1[:, None] * WqA).astype(bf)
    wk_d = (g1[:, None] * WkA).astype(bf)
    wv_d = (g1[:, None] * WvA).astype(bf)
    wo_d = np.asarray(Wo, np.float32).astype(bf)
    w1_d = (g2[:, None] * np.asarray(W1, np.float32)).astype(bf)
    w2_d = np.asarray(W2, np.float32).astype(bf)
    bq = b1v @ WqA
    bk = b1v @ WkA
    bqk = np.ascontiguousarray(
        np.stack([bq.reshape(NE, 128).T, bk.reshape(NE, 128).T], axis=1), np.float32)
    b1ff = b2v @ np.asarray(W1, np.float32)
    b1d = np.ascontiguousarray(b1ff.reshape(NM, 128).T, np.float32)
    idn = np.eye(128, dtype=bf)

    x = np.asarray(x, np.float32)
    in_maps = []
    for c in range(8):
        b, r = c // 2, c % 2
        xow = np.concatenate([x[b, s0:s1] for (s0, s1) in _own_slices(r)], axis=0)
        in_maps.append({
            "xg": np.ascontiguousarray(x[b]), "xo": np.ascontiguousarray(xow),
            "wq": wq_d, "wk": wk_d, "wv": wv_d, "wo": wo_d,
            "w1": w1_d, "w2": w2_d,
            "bqk": bqk, "b1d": b1d,
            "msk": _make_masks(r), "idn": idn,
        })
    return in_maps


def kernel(**inputs):
    if "prog" not in _prog_cache:
        _prog_cache["prog"] = _build_program()
    nc = _prog_cache["prog"]
    in_maps = _prep_core_inputs(**inputs)
    res = None
    last_err = None
    for attempt in range(3):
        try:
            res = run_bass_kernel_spmd(nc, in_maps, list(range(8)))
            break
        except Exception as e:  # transient device faults observed; retry
            last_err = e
            time.sleep(2.0)
    if res is None:
        raise last_err
    outs = res.results
    full = np.empty((B, S, E), np.float32)
    for c in range(8):
        b, r = c // 2, c % 2
        o = np.asarray(outs[c]["out"], np.float32)
        pos = 0
        for (s0, s1) in _own_slices(r):
            full[b, s0:s1] = o[pos:pos + (s1 - s0)]
            pos += s1 - s0
    return full



# revision 24
# speedup vs baseline: 1575.1393x; 1575.1393x over previous
"""Trainium2 Bass kernel for a dense transformer block (pre-LN, causal MHA + GELU FFN).

Sharding: 8 cores = 4 batches x 2 roles. Each core handles one batch.
The two cores of a batch split the 2048 queries in a zigzag: role 0 owns
blocks [0:512) and [1536:2048), role 1 owns [512:1536). Both cores
redundantly compute LN1 + K/V for all 2048 tokens of their batch, which
avoids all cross-core communication.

Everything on-device is FEATURE-major ([128 features, tokens]): the host
supplies x pre-transposed (a pure layout transform), LayerNorm statistics
are computed with ones-vector matmuls on the tensor engine followed by
row math + partition broadcasts, and the output is returned feature-major
(host untransposes). This eliminates all DMA transposes and DRAM spills
of the old token-major LayerNorm pipeline.

The two roles are compiled as two separate programs with EXACT causal
k-tile extents (role 0: 4+16, role 1: 8+12 k-tiles for its two query
chunks, 20 each) instead of one padded-uniform SPMD program (8+16); only
the 4 diagonal-band k-tiles per chunk apply a host-provided mask. The
two programs are dispatched concurrently on disjoint 4-device halves.
"""

import time

import numpy as np
import ml_dtypes

import concourse.bass as bass
import concourse.tile as tile
from concourse import bacc
from concourse import mybir
from concourse.bass_utils import run_bass_kernel_spmd

F32 = mybir.dt.float32
BF16 = mybir.dt.bfloat16
AF = mybir.ActivationFunctionType
OP = mybir.AluOpType

B, S, E, H, DH = 4, 2048, 1024, 16, 64
MFF = 6 * E            # 6144
SO = S // 2            # own tokens per core: 1024
LN_EPS = 1e-5
NT = S // 128          # 16 token tiles (global)
NTO = SO // 128        # 8 own token tiles
NE = E // 128          # 8 feature chunks
NM = MFF // 128        # 48 ffn chunks
_prog_cache = {}


def _build_program(role, stage=4, reps=1):
    nc = bacc.Bacc(None)
    # exact causal structure per role: query chunk qc covers q-tiles
    # [qt0, qt0+4); it needs qt0+4 k-tiles, of which only the last 4
    # (the diagonal band) require masking.
    qt0s = (0, 12) if role == 0 else (4, 8)

    xgt = nc.declare_dram_parameter("xgt", [128, NE, S], BF16, isOutput=False)
    xot = nc.declare_dram_parameter("xot", [128, NE, SO], BF16, isOutput=False)
    xof = nc.declare_dram_parameter("xof", [128, NE, SO], F32, isOutput=False)
    wq = nc.declare_dram_parameter("wq", [E, E], BF16, isOutput=False)
    wk = nc.declare_dram_parameter("wk", [E, E], BF16, isOutput=False)
    wv = nc.declare_dram_parameter("wv", [E, E], BF16, isOutput=False)
    wo = nc.declare_dram_parameter("wo", [E, E], BF16, isOutput=False)
    w1 = nc.declare_dram_parameter("w1", [E, MFF], BF16, isOutput=False)
    w2 = nc.declare_dram_parameter("w2", [MFF, E], BF16, isOutput=False)
    bqk = nc.declare_dram_parameter("bqk", [128, 2, NE], F32, isOutput=False)
    b1d = nc.declare_dram_parameter("b1d", [128, NM], F32, isOutput=False)
    msk = nc.declare_dram_parameter("msk", [128, 4, 1024], BF16, isOutput=False)
    out = nc.declare_dram_parameter("out", [128, NE, SO], F32, isOutput=True)

    with tile.TileContext(nc) as tc:
        def _body():
            # ---- kernel-wide pools ----
            gp = tc.alloc_tile_pool(name="gp", bufs=1)
            bqk_s = gp.tile([128, 2, NE], F32, tag="bqk")
            b1_s = gp.tile([128, NM], F32, tag="b1")
            eps_t = gp.tile([128, 1], F32, tag="eps")
            ones_t = gp.tile([128, 1], BF16, tag="ones")

            # right-side stack: CT2 outlives attention; ab/mp die with it
            ct2p = tc.alloc_tile_pool(name="ct2p", bufs=1, side="right")
            CT2 = ct2p.tile([128, NE, SO], BF16, tag="CT2")
            ab = tc.alloc_tile_pool(name="ab", bufs=1, side="right")
            KF = ab.tile([128, NE, S], BF16, tag="KF")
            QF = ab.tile([128, NE, SO], BF16, tag="QF")
            VT = ab.tile([128, NT, H * 65], BF16, tag="VT")
            mp = tc.alloc_tile_pool(name="mp", bufs=1, side="right")
            masks = mp.tile([128, 4, 1024], BF16, tag="masks")

            nc.sync.dma_start(out=masks, in_=msk[:, :, :])
            nc.sync.dma_start(out=bqk_s, in_=bqk[:, :, :])
            nc.sync.dma_start(out=b1_s, in_=b1d[:, :])
            nc.vector.memset(eps_t, LN_EPS)
            nc.vector.memset(ones_t, 1.0 / E)

            # ---- phase A: LN1 (feature-major) + Q/K/V projections ----
            xfp = tc.alloc_tile_pool(name="xfp", bufs=1)
            hF = xfp.tile([128, NE, S], BF16, tag="hF")  # starts as raw xgT
            bc = tc.alloc_tile_pool(name="bc", bufs=1)
            xop = tc.alloc_tile_pool(name="xop", bufs=1)
            hFq = xop.tile([128, NE, SO], BF16, tag="hFq")  # starts as raw xoT
            rows = tc.alloc_tile_pool(name="rows", bufs=3)
            sqp = tc.alloc_tile_pool(name="sqp", bufs=3)
            wvp = tc.alloc_tile_pool(name="wvp", bufs=1)
            wqk = tc.alloc_tile_pool(name="wqk", bufs=2)
            ps_st = tc.alloc_tile_pool(name="ps_st", bufs=4, space="PSUM")

            # load x feature-major (two halves each for pipelining)
            for hf in range(2):
                nc.sync.dma_start(out=hF[:, hf * 4:(hf + 1) * 4, :],
                                  in_=xgt[:, hf * 4:(hf + 1) * 4, :])
            nc.sync.dma_start(out=hFq, in_=xot[:, :, :])

            def ln_normalize(xT, ncols, pfx):
                """LayerNorm in feature-major layout, in place (xT becomes h).
                mu and E[x^2] rows come from ones-vector matmuls on PE
                (contraction over the 128 feature partitions, accumulated
                over e-chunks); row math on DVE/ACT; partition broadcast of
                rs and mu*rs (bf16); then x <- x*rs - mus per e-chunk."""
                nch = ncols // 512
                rsB = bc.tile([128, ncols], BF16, tag=f"rsB{pfx}")
                musB = bc.tile([128, ncols], BF16, tag=f"musB{pfx}")
                # stats in passes of 2 chunks (PSUM budget)
                for half in range(nch // 2):
                    cs = (half * 2, half * 2 + 1)
                    ps_mu = {c: ps_st.tile([1, 512], F32, tag="st",
                                           name=f"mu{pfx}{c}") for c in cs}
                    ps_sq = {c: ps_st.tile([1, 512], F32, tag="st",
                                           name=f"sq{pfx}{c}") for c in cs}
                    for e in range(NE):
                        for c in cs:
                            sq = sqp.tile([128, 512], BF16, tag="sq",
                                          name=f"sq{pfx}{e}_{c}")
                            xc = xT[:, e, c * 512:(c + 1) * 512]
                            nc.vector.tensor_tensor(out=sq, in0=xc, in1=xc,
                                                    op=OP.mult)
                            nc.tensor.matmul(ps_mu[c], ones_t, xc,
                                             start=(e == 0), stop=(e == NE - 1))
                            nc.tensor.matmul(ps_sq[c], ones_t, sq,
                                             start=(e == 0), stop=(e == NE - 1))
                    for c in cs:
                        mur = rows.tile([1, 512], F32, tag="row",
                                        name=f"mur{pfx}{c}")
                        sqr = rows.tile([1, 512], F32, tag="row",
                                        name=f"sqr{pfx}{c}")
                        tr = rows.tile([1, 512], F32, tag="row",
                                       name=f"tr{pfx}{c}")
                        nc.vector.tensor_copy(out=mur, in_=ps_mu[c])
                        nc.vector.tensor_copy(out=sqr, in_=ps_sq[c])
                        nc.vector.tensor_tensor(out=tr, in0=mur, in1=mur,
                                                op=OP.mult)
                        nc.vector.tensor_tensor(out=sqr, in0=sqr, in1=tr,
                                                op=OP.subtract)  # var
                        nc.scalar.activation(out=sqr, in_=sqr, func=AF.Sqrt,
                                             bias=eps_t[0:1, :], scale=1.0)
                        nc.vector.reciprocal(out=sqr, in_=sqr)  # rs (f32)
                        nc.vector.tensor_tensor(out=mur, in0=mur, in1=sqr,
                                                op=OP.mult)     # mu*rs
                        rsr = rows.tile([1, 512], BF16, tag="rowb",
                                        name=f"rsr{pfx}{c}")
                        msr = rows.tile([1, 512], BF16, tag="rowb",
                                        name=f"msr{pfx}{c}")
                        nc.vector.tensor_copy(out=rsr, in_=sqr)
                        nc.vector.tensor_copy(out=msr, in_=mur)
                        nc.gpsimd.partition_broadcast(
                            rsB[:, c * 512:(c + 1) * 512], rsr)
                        nc.gpsimd.partition_broadcast(
                            musB[:, c * 512:(c + 1) * 512], msr)
                # normalize in place: x <- x*rs - mus
                for e in range(NE):
                    xe = xT[:, e, :]
                    nc.vector.tensor_tensor(out=xe, in0=xe, in1=rsB, op=OP.mult)
                    nc.vector.tensor_tensor(out=xe, in0=xe, in1=musB,
                                            op=OP.subtract)

            ln_normalize(hF, S, "g")
            ln_normalize(hFq, SO, "o")

            ps_st.release()
            ps_mm = tc.alloc_tile_pool(name="ps_mm_a", bufs=4, space="PSUM")

            def proj_qk(w_dram, srcF, ntok, dstF, bias_col, pname):
                # stream W in quarter tiles [128, e, 256] (2 head-pairs each);
                # token range processed in passes of <=1024 cols (2 psum bufs)
                nch = ntok // 512
                for hp in range(NE):
                    if hp % 2 == 0:
                        wt = wqk.tile([128, NE, 256], BF16, tag="wqk",
                                      name=f"w{pname}{hp // 2}")
                        nc.sync.dma_start(
                            out=wt,
                            in_=w_dram[:, hp * 128:(hp + 2) * 128].rearrange(
                                "(e p) m -> p e m", p=128))
                    wcol = (hp % 2) * 128
                    for half in range(nch // 2):
                        cs = (half * 2, half * 2 + 1)
                        pss = {c: ps_mm.tile([128, 512], F32, tag="mm",
                                             name=f"{pname}{hp}_{c}")
                               for c in cs}
                        for e in range(NE):
                            for c in cs:
                                nc.tensor.matmul(
                                    pss[c], wt[:, e, wcol:wcol + 128],
                                    srcF[:, e, c * 512:(c + 1) * 512],
                                    start=(e == 0), stop=(e == NE - 1))
                        for c in cs:
                            nc.vector.tensor_scalar(
                                out=dstF[:, hp, c * 512:(c + 1) * 512],
                                in0=pss[c],
                                scalar1=bqk_s[:, bias_col, hp:hp + 1],
                                scalar2=None, op0=OP.add)

            proj_qk(wk, hF, S, KF, 1, "k")
            proj_qk(wq, hFq, SO, QF, 0, "q")

            # V projection: token-major with a ones column per head.
            # W_v streamed in halves of the output dim.
            VTv = VT.rearrange("p t (h c) -> p t h c", c=65)
            for c in range(2):
                wvt = wvp.tile([128, NE, 512], BF16, tag="wv", name=f"wv{c}")
                nc.sync.dma_start(
                    out=wvt,
                    in_=wv[:, c * 512:(c + 1) * 512].rearrange(
                        "(e p) m -> p e m", p=128))
                for t in range(NT):
                    if c == 0:
                        nc.vector.memset(VTv[:, t, :, 64:65], 1.0)
                    ps = ps_mm.tile([128, 512], F32, tag="mm", name=f"v{t}_{c}")
                    for e in range(NE):
                        nc.tensor.matmul(
                            ps, hF[:, e, t * 128:(t + 1) * 128],
                            wvt[:, e, :],
                            start=(e == 0), stop=(e == NE - 1))
                    nc.vector.tensor_copy(
                        out=VTv[:, t, 8 * c:8 * c + 8, 0:64],
                        in_=ps.rearrange("p (h c) -> p h c", c=64))

            ps_mm.release()
            wqk.release()
            wvp.release()
            sqp.release()
            rows.release()
            xop.release()
            bc.release()
            xfp.release()

            # ============ phase B: attention ============
            skipB = stage < 2
            skipC = stage < 3
            skipD = stage < 4
            if skipB:
                nc.vector.memset(CT2[:, :, :], 0.0)
            ptile = tc.alloc_tile_pool(name="ptile", bufs=4)
            small = tc.alloc_tile_pool(name="small", bufs=3)
            ps_sc = tc.alloc_tile_pool(name="ps_sc", bufs=3, space="PSUM")
            ps_ctx = tc.alloc_tile_pool(name="ps_ctx", bufs=2, space="PSUM")

            for hp in range(NE if not skipB else 0):
                for qc in range(2):
                    qt0 = qt0s[qc]
                    ng = qt0 // 2 + 2  # groups of 2 k-tiles (exact causal)
                    ctxs = (ps_ctx.tile([65, 512], F32, tag="ctx", name=f"cx{hp}_{qc}_0"),
                            ps_ctx.tile([65, 512], F32, tag="ctx", name=f"cx{hp}_{qc}_1"))
                    for g in range(ng):
                        # per-head [128,1024] score tiles (2 PSUM banks each)
                        # so matmuls of group g+1 overlap exp/mask of group g
                        for hh in range(2):
                            hoff = hh * 64
                            sc = ps_sc.tile([128, 1024], F32, tag="sc",
                                            name=f"sc{hp}_{qc}_{g}_{hh}")
                            for kl in range(2):
                                kt = g * 2 + kl
                                nc.tensor.matmul(
                                    sc[:, kl * 512:(kl + 1) * 512],
                                    KF[hoff:hoff + 64, hp, kt * 128:(kt + 1) * 128],
                                    QF[hoff:hoff + 64, hp, qc * 512:(qc + 1) * 512],
                                    start=True, stop=True)
                            pt = ptile.tile([128, 1024], BF16, tag="pt",
                                            name=f"pt{hp}_{qc}_{g}_{hh}")
                            nc.scalar.activation(out=pt, in_=sc, func=AF.Exp,
                                                 scale=0.125)
                            # only the diagonal band (last 2 groups =
                            # 4 k-tiles) needs masking; earlier k-tiles
                            # are fully allowed. Mask data is
                            # head-independent.
                            if g >= qt0 // 2:
                                slot = qc * 2 + (g - qt0 // 2)
                                nc.vector.tensor_tensor(
                                    out=pt, in0=pt,
                                    in1=masks[:, slot, :], op=OP.mult)
                            h = hp * 2 + hh
                            for kl in range(2):
                                kt = g * 2 + kl
                                nc.tensor.matmul(
                                    ctxs[hh], VTv[:, kt, h, :],
                                    pt[:, kl * 512:(kl + 1) * 512],
                                    start=(g == 0 and kl == 0),
                                    stop=(g == ng - 1 and kl == 1))
                    for hh in range(2):
                        rs1 = small.tile([1, 512], F32, tag="rs1", name=f"r{hp}_{qc}_{hh}")
                        nc.vector.reciprocal(out=rs1, in_=ctxs[hh][64:65, :])
                        rsb = small.tile([64, 512], F32, tag="rsb", name=f"rb{hp}_{qc}_{hh}")
                        nc.gpsimd.partition_broadcast(rsb, rs1)
                        nc.vector.tensor_tensor(
                            out=CT2[hh * 64:hh * 64 + 64, hp, qc * 512:(qc + 1) * 512],
                            in0=ctxs[hh][0:64, :], in1=rsb, op=OP.mult)

            ps_ctx.release()
            ps_sc.release()
            small.release()
            ptile.release()
            mp.release()
            ab.release()

            # ============ phase C: Wo + residual (feature-major), LN2, FFN1 ====
            x2p = tc.alloc_tile_pool(name="x2p", bufs=1)
            X2F = x2p.tile([128, NE, SO], F32, tag="X2F")
            wop = tc.alloc_tile_pool(name="wop", bufs=1)
            wo_s = wop.tile([128, NE, E], BF16, tag="wo")
            xo2p = tc.alloc_tile_pool(name="xo2p", bufs=1)
            xo_f = xo2p.tile([128, NE, SO], F32, tag="xof")
            ps_mm2 = tc.alloc_tile_pool(name="ps_mm_c", bufs=4, space="PSUM")

            nc.sync.dma_start(out=wo_s, in_=wo.rearrange("(e p) m -> p e m", p=128))
            nc.sync.dma_start(out=xo_f, in_=xof[:, :, :])
            for e2 in range(NE):
                for tcn in range(2):
                    ps = ps_mm2.tile([128, 512], F32, tag="mm", name=f"o{e2}_{tcn}")
                    for hp in range(NE):
                        nc.tensor.matmul(
                            ps, wo_s[:, hp, e2 * 128:(e2 + 1) * 128],
                            CT2[:, hp, tcn * 512:(tcn + 1) * 512],
                            start=(hp == 0), stop=(hp == NE - 1))
                    nc.vector.tensor_tensor(
                        out=X2F[:, e2, tcn * 512:(tcn + 1) * 512], in0=ps,
                        in1=xo_f[:, e2, tcn * 512:(tcn + 1) * 512], op=OP.add)

            xo2p.release()
            wop.release()
            ct2p.release()

            # LN2 feature-major: cast X2F -> bf16 into h2F, normalize in place
            h2p = tc.alloc_tile_pool(name="h2p", bufs=1)
            h2F = h2p.tile([128, NE, SO], BF16, tag="h2F")
            bc2 = tc.alloc_tile_pool(name="bc2", bufs=1)
            rows2 = tc.alloc_tile_pool(name="rows2", bufs=3)
            sqp2 = tc.alloc_tile_pool(name="sqp2", bufs=3)
            ps_st2 = tc.alloc_tile_pool(name="ps_st2", bufs=4, space="PSUM")
            if skipC:
                nc.vector.memset(h2F[:, :, :], 0.0)
            else:
                for e in range(NE):
                    nc.vector.tensor_copy(out=h2F[:, e, :], in_=X2F[:, e, :])
                rsB = bc2.tile([128, SO], BF16, tag="rsB2")
                musB = bc2.tile([128, SO], BF16, tag="musB2")
                cs = (0, 1)
                ps_mu = {c: ps_st2.tile([1, 512], F32, tag="st", name=f"mu2{c}")
                         for c in cs}
                ps_sq = {c: ps_st2.tile([1, 512], F32, tag="st", name=f"sq2{c}")
                         for c in cs}
                for e in range(NE):
                    for c in cs:
                        sq = sqp2.tile([128, 512], BF16, tag="sq",
                                       name=f"sq2{e}_{c}")
                        xc = h2F[:, e, c * 512:(c + 1) * 512]
                        nc.vector.tensor_tensor(out=sq, in0=xc, in1=xc, op=OP.mult)
                        nc.tensor.matmul(ps_mu[c], ones_t, xc,
                                         start=(e == 0), stop=(e == NE - 1))
                        nc.tensor.matmul(ps_sq[c], ones_t, sq,
                                         start=(e == 0), stop=(e == NE - 1))
                for c in cs:
                    mur = rows2.tile([1, 512], F32, tag="row", name=f"mur2{c}")
                    sqr = rows2.tile([1, 512], F32, tag="row", name=f"sqr2{c}")
                    tr = rows2.tile([1, 512], F32, tag="row", name=f"tr2{c}")
                    nc.vector.tensor_copy(out=mur, in_=ps_mu[c])
                    nc.vector.tensor_copy(out=sqr, in_=ps_sq[c])
                    nc.vector.tensor_tensor(out=tr, in0=mur, in1=mur, op=OP.mult)
                    nc.vector.tensor_tensor(out=sqr, in0=sqr, in1=tr,
                                            op=OP.subtract)
                    nc.scalar.activation(out=sqr, in_=sqr, func=AF.Sqrt,
                                         bias=eps_t[0:1, :], scale=1.0)
                    nc.vector.reciprocal(out=sqr, in_=sqr)
                    nc.vector.tensor_tensor(out=mur, in0=mur, in1=sqr, op=OP.mult)
                    rsr = rows2.tile([1, 512], BF16, tag="rowb", name=f"rsr2{c}")
                    msr = rows2.tile([1, 512], BF16, tag="rowb", name=f"msr2{c}")
                    nc.vector.tensor_copy(out=rsr, in_=sqr)
                    nc.vector.tensor_copy(out=msr, in_=mur)
                    nc.gpsimd.partition_broadcast(
                        rsB[:, c * 512:(c + 1) * 512], rsr)
                    nc.gpsimd.partition_broadcast(
                        musB[:, c * 512:(c + 1) * 512], msr)
                for e in range(NE):
                    xe = h2F[:, e, :]
                    nc.vector.tensor_tensor(out=xe, in0=xe, in1=rsB, op=OP.mult)
                    nc.vector.tensor_tensor(out=xe, in0=xe, in1=musB,
                                            op=OP.subtract)

            ps_st2.release()
            sqp2.release()
            rows2.release()
            bc2.release()

            hidp = tc.alloc_tile_pool(name="hidp", bufs=1)
            HID = hidp.tile([128, NM, SO], BF16, tag="HID")
            w1p = tc.alloc_tile_pool(name="w1p", bufs=2)
            MQ = NM // 8  # 6 mo-chunks per eighth-load of W1
            for mq in range(8 if not skipC else 0):
                w1t = w1p.tile([128, NE, MQ * 128], BF16, tag="w1t", name=f"w1t{mq}")
                nc.sync.dma_start(
                    out=w1t,
                    in_=w1[:, mq * MQ * 128:(mq + 1) * MQ * 128].rearrange(
                        "(e p) m -> p e m", p=128))
                for mi in range(MQ):
                    mo = mq * MQ + mi
                    for c in range(2):
                        ps = ps_mm2.tile([128, 512], F32, tag="mm", name=f"h{mo}_{c}")
                        for e in range(NE):
                            nc.tensor.matmul(
                                ps, w1t[:, e, mi * 128:(mi + 1) * 128],
                                h2F[:, e, c * 512:(c + 1) * 512],
                                start=(e == 0), stop=(e == NE - 1))
                        nc.scalar.activation(
                            out=HID[:, mo, c * 512:(c + 1) * 512], in_=ps,
                            func=AF.Gelu, bias=b1_s[:, mo:mo + 1], scale=1.0)

            w1p.release()
            ps_mm2.release()

            # ============ phase D: FFN down + residual + store (feature-major) ==
            w2p = tc.alloc_tile_pool(name="w2p", bufs=2)
            outp = tc.alloc_tile_pool(name="outp", bufs=4)
            ps_f2 = tc.alloc_tile_pool(name="ps_f2", bufs=4, space="PSUM")
            if skipC or skipD:
                nc.vector.memset(HID[:, :, :], 0.0)
            nmw = NM if not skipD else 1
            for e2 in range(NE):
                w2t = w2p.tile([128, NM, 128], BF16, tag="w2t", name=f"w2t{e2}")
                nc.sync.dma_start(
                    out=w2t,
                    in_=w2[:, e2 * 128:(e2 + 1) * 128].rearrange(
                        "(m p) e -> p m e", p=128))
                for tcn in range(2):
                    ps = ps_f2.tile([128, 512], F32, tag="f2", name=f"f{e2}_{tcn}")
                    for m in range(nmw):
                        nc.tensor.matmul(
                            ps, w2t[:, m, :],
                            HID[:, m, tcn * 512:(tcn + 1) * 512],
                            start=(m == 0), stop=(m == NM - 1))
                    ot = outp.tile([128, 512], F32, tag="ot", name=f"ot{e2}_{tcn}")
                    nc.vector.tensor_tensor(
                        out=ot, in0=ps,
                        in1=X2F[:, e2, tcn * 512:(tcn + 1) * 512], op=OP.add)
                    nc.sync.dma_start(
                        out=out[:, e2, tcn * 512:(tcn + 1) * 512], in_=ot)

            ps_f2.release()
            outp.release()
            w2p.release()
            hidp.release()
            h2p.release()
            x2p.release()
            gp.release()

        for _rep in range(reps):
            _body()

    nc.compile()
    return nc


def _own_slices(role):
    if role == 0:
        return [(0, 512), (1536, 2048)]
    return [(512, 1024), (1024, 1536)]


def _make_masks(role):
    """[128, 4, 1024] bf16; slot = qc*2 + pair-in-diagonal-band
    (head-independent). The band is k-tiles [qt0, qt0+4) for the query
    chunk starting at q-tile qt0."""
    qt0s = (0, 12) if role == 0 else (4, 8)
    m = np.zeros((128, 4, 1024), np.float32)
    ki = np.arange(128)[:, None]
    qi = np.arange(512)[None, :]
    for qc in range(2):
        qt0 = qt0s[qc]
        qs = qt0 * 128
        for sub in range(2):
            slot = qc * 2 + sub
            for kl in range(2):
                kt = qt0 + sub * 2 + kl
                blk = ((kt * 128 + ki) <= (qs + qi))
                m[:, slot, kl * 512:(kl + 1) * 512] = blk
    return m.astype(ml_dtypes.bfloat16)


def _featmajor(xt):
    """[T, E] -> [128, NE, T]: out[p, e, t] = xt[t, e*128+p]."""
    T = xt.shape[0]
    return np.ascontiguousarray(xt.T.reshape(NE, 128, T).transpose(1, 0, 2))


def _prep_core_inputs(x, Wq, Wk, Wv, Wo, W1, W2, ln1_g, ln1_b, ln2_g, ln2_b):
    bf = ml_dtypes.bfloat16
    WqA = np.transpose(np.asarray(Wq, np.float32), (1, 0, 2)).reshape(E, E)
    WkA = np.transpose(np.asarray(Wk, np.float32), (1, 0, 2)).reshape(E, E)
    WvA = np.transpose(np.asarray(Wv, np.float32), (1, 0, 2)).reshape(E, E)
    g1 = np.asarray(ln1_g, np.float32)
    b1v = np.asarray(ln1_b, np.float32)
    g2 = np.asarray(ln2_g, np.float32)
    b2v = np.asarray(ln2_b, np.float32)
    assert np.all(b1v == 0.0), "nonzero ln1 bias unsupported (V bias path)"
    wq_d = (g1[:, None] * WqA).astype(bf)
    wk_d = (g1[:, None] * WkA).astype(bf)
    wv_d = (g1[:, None] * WvA).astype(bf)
    wo_d = np.asarray(Wo, np.float32).astype(bf)
    w1_d = (g2[:, None] * np.asarray(W1, np.float32)).astype(bf)
    w2_d = np.asarray(W2, np.float32).astype(bf)
    bq = b1v @ WqA
    bk = b1v @ WkA
    bqk = np.ascontiguousarray(
        np.stack([bq.reshape(NE, 128).T, bk.reshape(NE, 128).T], axis=1), np.float32)
    b1ff = b2v @ np.asarray(W1, np.float32)
    b1d = np.ascontiguousarray(b1ff.reshape(NM, 128).T, np.float32)

    x = np.asarray(x, np.float32)
    in_maps = []
    for c in range(8):
        b, r = c // 2, c % 2
        xow = np.concatenate([x[b, s0:s1] for (s0, s1) in _own_slices(r)], axis=0)
        xowf = _featmajor(xow)
        in_maps.append({
            "xgt": _featmajor(x[b]).astype(bf),
            "xot": xowf.astype(bf),
            "xof": xowf,
            "wq": wq_d, "wk": wk_d, "wv": wv_d, "wo": wo_d,
            "w1": w1_d, "w2": w2_d,
            "bqk": bqk, "b1d": b1d,
            "msk": _make_masks(r),
        })
    return in_maps


def _make_runner(nc, n_cores, dev_lo):
    """Build a reusable jitted SPMD callable for `nc` on jax devices
    [dev_lo, dev_lo+n_cores). Mirrors bass2jax.run_bass_via_pjrt but holds
    the jit (no per-call retrace) and allows a device offset so two
    different programs can run concurrently on disjoint device halves."""
    import jax
    from jax.sharding import Mesh, PartitionSpec
    from jax.experimental.shard_map import shard_map
    from concourse.bass2jax import (
        _bass_exec_p, install_neuronx_cc_hook, partition_id_tensor)

    install_neuronx_cc_hook()
    partition_name = nc.partition_id_tensor.name if nc.partition_id_tensor else None
    in_names, out_names, out_avals, zero_outs = [], [], [], []
    for alloc in nc.m.functions[0].allocations:
        if not isinstance(alloc, mybir.MemoryLocationSet):
            continue
        name = alloc.memorylocations[0].name
        if alloc.kind == "ExternalInput":
            if name != partition_name:
                in_names.append(name)
        elif alloc.kind == "ExternalOutput":
            shape = tuple(alloc.tensor_shape)
            dtype = mybir.dt.np(alloc.dtype)
            out_avals.append(jax.core.ShapedArray(shape, dtype))
            out_names.append(name)
            zero_outs.append(np.zeros(shape, dtype))
    n_params = len(in_names)
    n_outs = len(out_avals)
    in_names_all = in_names + out_names + ([partition_name] if partition_name else [])

    def _bdy(*args):
        operands = list(args)
        if partition_name is not None:
            operands.append(partition_id_tensor())
        outs = _bass_exec_p.bind(
            *operands, out_avals=tuple(out_avals), in_names=tuple(in_names_all),
            out_names=tuple(out_names), lowering_input_output_aliases=(),
            sim_require_finite=True, sim_require_nnan=True, nc=nc)
        return tuple(outs)

    devices = jax.devices()[dev_lo:dev_lo + n_cores]
    mesh = Mesh(np.asarray(devices), ("core",))
    sharded = jax.jit(
        shard_map(_bdy, mesh=mesh,
                  in_specs=(PartitionSpec("core"),) * (n_params + n_outs),
                  out_specs=(PartitionSpec("core"),) * n_outs,
                  check_rep=False),
        donate_argnums=tuple(range(n_params, n_params + n_outs)),
        keep_unused=True)

    def call(in_maps):
        concat_in = [
            np.concatenate([np.asarray(in_maps[c][nm]) for c in range(n_cores)],
                           axis=0)
            for nm in in_names
        ]
        concat_zeros = [np.zeros((n_cores * z.shape[0], *z.shape[1:]), z.dtype)
                        for z in zero_outs]
        out_arrs = sharded(*concat_in, *concat_zeros)

        def finish():
            return [
                {nm: np.asarray(out_arrs[i]).reshape(
                    n_cores, *out_avals[i].shape)[c]
                 for i, nm in enumerate(out_names)}
                for c in range(n_cores)
            ]
        return finish

    return call


def kernel(**inputs):
    if "runners" not in _prog_cache:
        ncs = [_build_program(role) for role in range(2)]
        _prog_cache["runners"] = [
            _make_runner(ncs[role], 4, role * 4) for role in range(2)
        ]
    runners = _prog_cache["runners"]
    in_maps = _prep_core_inputs(**inputs)
    maps_by_role = [[in_maps[b * 2 + r] for b in range(B)] for r in range(2)]
    outs_by_role = None
    last_err = None
    for attempt in range(3):
        try:
            # dispatch both role-programs; they run concurrently on
            # disjoint 4-device halves (jax dispatch is async)
            fin = [runners[r](maps_by_role[r]) for r in range(2)]
            outs_by_role = [f() for f in fin]
            break
        except Exception as e:  # transient device faults observed; retry
            last_err = e
            time.sleep(2.0)
    if outs_by_role is None:
        raise last_err
    full = np.empty((B, S, E), np.float32)
    for r in range(2):
        for b in range(B):
            o = np.asarray(outs_by_role[r][b]["out"], np.float32)  # [128,NE,SO]
            o = o.transpose(2, 1, 0).reshape(SO, E)  # token-major [SO, E]
            pos = 0
            for (s0, s1) in _own_slices(r):
                full[b, s0:s1] = o[pos:pos + (s1 - s0)]
                pos += s1 - s0
    return full
